# revision 1
# baseline (speedup 1.0000x reference)
"""Mask R-CNN paste_masks_in_image kernel for Trainium2 (8 NeuronCores).

out[n] = Y_n @ mask_n @ X_n  (separable bilinear paste, f32)

 - X_n [28, img_w] column-interp matrix, Y_n [img_h, 28] row-interp matrix
   (<=2 nonzeros per output row/col) are built on host from boxes.
 - Device (per core, 16 instances): mx = maskT.T @ X on TensorE (f32r),
   then rows in 3 permuted 128-row matmuls; only a 384-row full-width
   window per instance is written to HBM via one indirect scatter DMA
   (per-partition row triplets are DRAM-contiguous). Rows outside the
   window are never written: the runner pre-zeros/donates output buffers.
 - Falls back to a dense full-image writer if any box window exceeds the
   static 384-row budget (cannot happen for in-distribution inputs).
"""
import sys

if "/opt/trn_rl_repo" not in sys.path:
    sys.path.insert(0, "/opt/trn_rl_repo")

import numpy as np

N_CORES = 8
HM = WM = 28
PW = 112          # payload partitions
WIN = 3 * PW      # 336-row window; max nonzero span is <=309 rows

_BUILD_CACHE = {}
_ws_ctr = [0]


def _split_multi_waits(nc):
    """This image's walrus allows only ONE sync-wait per instruction; hoist
    extra waits onto preceding NoOps on the same engine."""
    import concourse.mybir as mybir

    for fn in nc.m.functions:
        for blk in fn.blocks:
            insts = list(blk.instructions)
            out = []
            changed = False
            for inst in insts:
                si = getattr(inst, "sync_info", None)
                waits = list(si.on_wait) if (si is not None and si.on_wait) else []
                if len(waits) > 1:
                    changed = True
                    for w in waits[:-1]:
                        _ws_ctr[0] += 1
                        out.append(
                            mybir.InstNoOp(
                                name=f"waitsplit-{_ws_ctr[0]}",
                                engine=inst.engine,
                                sync_info=mybir.SyncInfo(on_wait=[w], on_update=[]),
                            )
                        )
                    si.on_wait = [waits[-1]]
                out.append(inst)
            if changed:
                try:
                    blk.instructions = out
                except Exception:
                    del blk.instructions[:]
                    blk.instructions.extend(out)


def _interp_mats(p0, p1, out_size, mask_size):
    """W[n, k, j] = w0*(i0==k) + w1*(i0+1==k); exact f32 replication of the
    reference's align_corners=False bilinear weights with zero padding."""
    xs = (np.arange(out_size, dtype=np.float32) + np.float32(0.5))[None, :]
    g = (xs - p0[:, None]) / (p1 - p0)[:, None] * np.float32(2) - np.float32(1)
    p = (g + np.float32(1)) * np.float32(mask_size * 0.5) - np.float32(0.5)
    f = np.floor(p)
    i0 = f.astype(np.int64)
    w1 = (p - f).astype(np.float32)
    w0 = np.float32(1.0) - w1
    ks = np.arange(mask_size, dtype=np.int64)[None, :, None]
    W = (i0[:, None, :] == ks) * w0[:, None, :] + ((i0 + 1)[:, None, :] == ks) * w1[
        :, None, :
    ]
    return np.ascontiguousarray(W.astype(np.float32))


def _scaled_boxes(boxes, img_h, img_w, in_h, in_w):
    sx = np.float32(img_w / in_w)
    sy = np.float32(img_h / in_h)
    b = boxes.astype(np.float32) * np.array([sx, sy, sx, sy], np.float32)
    x0 = np.clip(b[:, 0], np.float32(0.0), np.float32(img_w))
    y0 = np.clip(b[:, 1], np.float32(0.0), np.float32(img_h))
    x1 = np.clip(b[:, 2], np.float32(0.0), np.float32(img_w))
    y1 = np.clip(b[:, 3], np.float32(0.0), np.float32(img_h))
    return x0, y0, x1, y1


def _chunks(img_w):
    out = []
    c = 0
    while c < img_w:
        cw = min(512, img_w - c)
        out.append((c, cw))
        c += cw
    return out


def _build_windowed(ni, img_h, img_w):
    import concourse.bass as bass
    import concourse.mybir as mybir
    from concourse.tile import TileContext

    f32 = mybir.dt.float32
    f32r = mybir.dt.float32r
    i32 = mybir.dt.int32
    nc = bass.Bass()
    maskT_d = nc.dram_tensor("maskT", [ni, WM, HM], f32r, kind="ExternalInput")
    x_d = nc.dram_tensor("xmat", [ni, WM, img_w], f32r, kind="ExternalInput")
    ytw_d = nc.dram_tensor("ytwmat", [ni, HM, WIN], f32r, kind="ExternalInput")
    idx_d = nc.dram_tensor("rowidx", [128, ni], i32, kind="ExternalInput")
    out_d = nc.dram_tensor("out", [ni, img_h, img_w], f32, kind="ExternalOutput")
    outv = out_d.rearrange("n h w -> (n h) w")
    chunks = _chunks(img_w)

    with TileContext(nc) as tc:
        with (
            tc.tile_pool(name="w", bufs=4) as wp,
            tc.tile_pool(name="ix", bufs=1) as ixp,
            tc.tile_pool(name="mx", bufs=2) as mxp,
            tc.tile_pool(name="psA", bufs=2, space="PSUM") as psa,
            tc.tile_pool(name="psB", bufs=2, space="PSUM") as psb,
            tc.tile_pool(name="pay", bufs=6) as payp,
        ):
            idxs = ixp.tile([128, ni], i32, tag="idx")
            nc.sync.dma_start(out=idxs[:], in_=idx_d[:])
            for n in range(ni):
                mT = wp.tile([WM, HM], f32r, tag="mT")
                xt = wp.tile([WM, img_w], f32r, tag="xt")
                ytw = wp.tile([HM, WIN], f32r, tag="ytw")
                nc.sync.dma_start(out=mT[:], in_=maskT_d[n])
                nc.sync.dma_start(out=xt[:], in_=x_d[n])
                nc.sync.dma_start(out=ytw[:], in_=ytw_d[n])

                mx = mxp.tile([HM, img_w], f32r, tag="mx")
                for j, (c0, cw) in enumerate(chunks):
                    pa = psa.tile([HM, 512], f32, tag="pa")
                    nc.tensor.matmul(
                        out=pa[:, :cw],
                        lhsT=mT[:],
                        rhs=xt[:, c0 : c0 + cw],
                        start=True,
                        stop=True,
                    )
                    if j % 2 == 0:
                        nc.vector.tensor_copy(out=mx[:, c0 : c0 + cw], in_=pa[:, :cw])
                    else:
                        nc.scalar.copy(out=mx[:, c0 : c0 + cw], in_=pa[:, :cw])

                pay = payp.tile([PW, 3 * img_w], f32, tag="pay")
                for j in range(3):
                    pb = psb.tile([PW, 3 * 512], f32, tag="pb")
                    for k, (c0, cw) in enumerate(chunks):
                        nc.tensor.matmul(
                            out=pb[:, k * 512 : k * 512 + cw],
                            lhsT=ytw[:, j * PW : (j + 1) * PW],
                            rhs=mx[:, c0 : c0 + cw],
                            start=True,
                            stop=True,
                        )
                    for k, (c0, cw) in enumerate(chunks):
                        eng = (
                            nc.vector.tensor_copy if (k + j) % 2 == 0 else nc.scalar.copy
                        )
                        eng(
                            out=pay[:, j * img_w + c0 : j * img_w + c0 + cw],
                            in_=pb[:, k * 512 : k * 512 + cw],
                        )
                nc.gpsimd.indirect_dma_start(
                    out=outv[:],
                    out_offset=bass.IndirectOffsetOnAxis(ap=idxs[:PW, n : n + 1], axis=0),
                    in_=pay[:],
                    in_offset=None,
                )
    _split_multi_waits(nc)
    return nc


def _build_dense(ni, img_h, img_w):
    """Fallback: writes every output pixel (no window assumption)."""
    import concourse.bass as bass
    import concourse.mybir as mybir
    from concourse.tile import TileContext

    f32 = mybir.dt.float32
    f32r = mybir.dt.float32r
    nc = bass.Bass()
    maskT_d = nc.dram_tensor("maskT", [ni, WM, HM], f32r, kind="ExternalInput")
    x_d = nc.dram_tensor("xmat", [ni, WM, img_w], f32r, kind="ExternalInput")
    yt_d = nc.dram_tensor("ytmat", [ni, HM, img_h], f32r, kind="ExternalInput")
    out_d = nc.dram_tensor("out", [ni, img_h, img_w], f32, kind="ExternalOutput")
    chunks = _chunks(img_w)
    rtiles = []
    r = 0
    while r < img_h:
        rh = min(128, img_h - r)
        rtiles.append((r, rh))
        r += rh

    with TileContext(nc) as tc:
        with (
            tc.tile_pool(name="w", bufs=3) as wp,
            tc.tile_pool(name="mx", bufs=2) as mxp,
            tc.tile_pool(name="psA", bufs=2, space="PSUM") as psa,
            tc.tile_pool(name="psB", bufs=2, space="PSUM") as psb,
            tc.tile_pool(name="ob", bufs=4) as obp,
        ):
            for n in range(ni):
                mT = wp.tile([WM, HM], f32r, tag="mT")
                xt = wp.tile([WM, img_w], f32r, tag="xt")
                yt = wp.tile([HM, img_h], f32r, tag="yt")
                nc.sync.dma_start(out=mT[:], in_=maskT_d[n])
                nc.sync.dma_start(out=xt[:], in_=x_d[n])
                nc.sync.dma_start(out=yt[:], in_=yt_d[n])

                mx = mxp.tile([HM, img_w], f32r, tag="mx")
                for j, (c0, cw) in enumerate(chunks):
                    pa = psa.tile([HM, 512], f32, tag="pa")
                    nc.tensor.matmul(
                        out=pa[:, :cw], lhsT=mT[:], rhs=xt[:, c0 : c0 + cw],
                        start=True, stop=True,
                    )
                    if j % 2 == 0:
                        nc.vector.tensor_copy(out=mx[:, c0 : c0 + cw], in_=pa[:, :cw])
                    else:
                        nc.scalar.copy(out=mx[:, c0 : c0 + cw], in_=pa[:, :cw])

                for r0, rh in rtiles:
                    pb = psb.tile([128, 3 * 512], f32, tag="pb")
                    for k, (c0, cw) in enumerate(chunks):
                        nc.tensor.matmul(
                            out=pb[:rh, k * 512 : k * 512 + cw],
                            lhsT=yt[:, r0 : r0 + rh],
                            rhs=mx[:, c0 : c0 + cw],
                            start=True, stop=True,
                        )
                    ob = obp.tile([128, img_w], f32, tag="ob")
                    for k, (c0, cw) in enumerate(chunks):
                        eng = nc.vector.tensor_copy if k % 2 == 0 else nc.scalar.copy
                        eng(out=ob[:rh, c0 : c0 + cw], in_=pb[:rh, k * 512 : k * 512 + cw])
                    nc.sync.dma_start(out=out_d[n, r0 : r0 + rh, :], in_=ob[:rh, :])
    _split_multi_waits(nc)
    return nc


def _prep_common(masks, boxes, img_h, img_w, in_h, in_w):
    x0, y0, x1, y1 = _scaled_boxes(boxes, img_h, img_w, in_h, in_w)
    xmat = _interp_mats(x0, x1, img_w, WM)   # [N, 28, img_w]
    ytmat = _interp_mats(y0, y1, img_h, HM)  # [N, 28, img_h]
    maskt = np.ascontiguousarray(np.transpose(masks[:, 0].astype(np.float32), (0, 2, 1)))
    return maskt, xmat, ytmat


def _windows(ytmat, img_h):
    """Per-instance window start r0 from the actual Yt nonzero columns.
    Returns (r0s, ok): ok False if any instance's span exceeds WIN."""
    n = ytmat.shape[0]
    nz = ytmat.any(axis=1)
    r0s = np.zeros(n, np.int64)
    for i in range(n):
        nzr = np.flatnonzero(nz[i])
        if nzr.size == 0:
            r0s[i] = 0
            continue
        r0 = min(max(int(nzr[0]), 0), max(img_h - WIN, 0))
        if int(nzr[-1]) >= r0 + WIN:
            return r0s, False
        r0s[i] = r0
    return r0s, True


def _run(masks, boxes, img_h, img_w, in_h, in_w, trace=False):
    from concourse.bass_utils import run_bass_kernel_spmd

    n = masks.shape[0]
    assert n % N_CORES == 0
    ni = n // N_CORES
    maskt, xmat, ytmat = _prep_common(masks, boxes, img_h, img_w, in_h, in_w)
    r0s, windowed = _windows(ytmat, img_h)
    windowed = windowed and img_h >= WIN

    if windowed:
        key = ("win", ni, img_h, img_w)
        if key not in _BUILD_CACHE:
            _BUILD_CACHE[key] = _build_windowed(ni, img_h, img_w)
        nc = _BUILD_CACHE[key]
        ytw = np.zeros((n, HM, WIN), np.float32)
        for i in range(n):
            w = ytmat[i][:, r0s[i] : r0s[i] + WIN]
            ytw[i] = np.concatenate([w[:, 0::3], w[:, 1::3], w[:, 2::3]], axis=1)
        in_maps = []
        for c in range(N_CORES):
            s = slice(c * ni, (c + 1) * ni)
            loc = np.arange(ni)
            idx = (
                (loc[None, :] * img_h + r0s[s][None, :]) + 3 * np.arange(128)[:, None]
            ).astype(np.int32)
            idx[PW:] = 0
            in_maps.append(
                {
                    "maskT": maskt[s],
                    "xmat": xmat[s],
                    "ytwmat": ytw[s],
                    "rowidx": np.ascontiguousarray(idx),
                }
            )
    else:
        key = ("dense", ni, img_h, img_w)
        if key not in _BUILD_CACHE:
            _BUILD_CACHE[key] = _build_dense(ni, img_h, img_w)
        nc = _BUILD_CACHE[key]
        in_maps = []
        for c in range(N_CORES):
            s = slice(c * ni, (c + 1) * ni)
            in_maps.append({"maskT": maskt[s], "xmat": xmat[s], "ytmat": ytmat[s]})

    res = run_bass_kernel_spmd(nc, in_maps, core_ids=list(range(N_CORES)), trace=trace)
    out = np.concatenate([res.results[c]["out"] for c in range(N_CORES)], axis=0)
    return out, res


def kernel(masks, boxes, img_h, img_w, in_h, in_w):
    img_h, img_w, in_h, in_w = int(img_h), int(img_w), int(in_h), int(in_w)
    masks = np.asarray(masks, dtype=np.float32)
    boxes = np.asarray(boxes, dtype=np.float32)
    out, _ = _run(masks, boxes, img_h, img_w, in_h, in_w, trace=False)
    return out



# revision 6
# speedup vs baseline: 2.9287x; 2.9287x over previous
"""Mask R-CNN paste_masks_in_image kernel for Trainium2 (8 NeuronCores).

out[n] = Y_n @ mask_n @ X_n  (separable bilinear paste, f32)

Fast path (windowed, variable row budget): host folds
W2_n = (Y_n @ M_n) over the instance's row window and slices X_n to a
512-col window. Instances are sorted by row span and dealt round-robin
so all 8 cores share one slot->row-budget pattern (b_s blocks of 128
rows, b_s in {1,2,3}); the budgets are baked into the compiled kernel
(cache key). Per slot the device does b_s matmuls [28x128]x[28x512]
and writes the [128*b_s, 512] patch with ONE kv_writeback DMA whose
int32 ctx index carries the dynamic flat offset r0*img_w + c0. Rows and
cols outside windows are never written: the runner pre-zeros/donates
output buffers.

Falls back to a dense full-image writer if any window exceeds the
static budgets (cannot happen for in-distribution inputs).
"""
import sys

if "/opt/trn_rl_repo" not in sys.path:
    sys.path.insert(0, "/opt/trn_rl_repo")

import numpy as np

N_CORES = 8
HM = WM = 28
RB = 128          # rows per block (= partitions per matmul)
MAXB = 3          # max blocks per slot -> max row span 384
WX = 512          # column window (kv_writeback ncn must be pow2 or <256)

_BUILD_CACHE = {}
_ws_ctr = [0]


def _split_multi_waits(nc):
    """This image's walrus allows only ONE sync-wait per instruction; hoist
    extra waits onto preceding NoOps on the same engine."""
    import concourse.mybir as mybir

    for fn in nc.m.functions:
        for blk in fn.blocks:
            insts = list(blk.instructions)
            out = []
            changed = False
            for inst in insts:
                si = getattr(inst, "sync_info", None)
                waits = list(si.on_wait) if (si is not None and si.on_wait) else []
                if len(waits) > 1:
                    changed = True
                    for w in waits[:-1]:
                        _ws_ctr[0] += 1
                        out.append(
                            mybir.InstNoOp(
                                name=f"waitsplit-{_ws_ctr[0]}",
                                engine=inst.engine,
                                sync_info=mybir.SyncInfo(on_wait=[w], on_update=[]),
                            )
                        )
                    si.on_wait = [waits[-1]]
                out.append(inst)
            if changed:
                try:
                    blk.instructions = out
                except Exception:
                    del blk.instructions[:]
                    blk.instructions.extend(out)


def _interp_mats(p0, p1, out_size, mask_size):
    """W[n, k, j] = w0*(i0==k) + w1*(i0+1==k); exact f32 replication of the
    reference's align_corners=False bilinear weights with zero padding."""
    xs = (np.arange(out_size, dtype=np.float32) + np.float32(0.5))[None, :]
    g = (xs - p0[:, None]) / (p1 - p0)[:, None] * np.float32(2) - np.float32(1)
    p = (g + np.float32(1)) * np.float32(mask_size * 0.5) - np.float32(0.5)
    f = np.floor(p)
    i0 = f.astype(np.int64)
    w1 = (p - f).astype(np.float32)
    w0 = np.float32(1.0) - w1
    ks = np.arange(mask_size, dtype=np.int64)[None, :, None]
    W = (i0[:, None, :] == ks) * w0[:, None, :] + ((i0 + 1)[:, None, :] == ks) * w1[
        :, None, :
    ]
    return np.ascontiguousarray(W.astype(np.float32))


def _scaled_boxes(boxes, img_h, img_w, in_h, in_w):
    sx = np.float32(img_w / in_w)
    sy = np.float32(img_h / in_h)
    b = boxes.astype(np.float32) * np.array([sx, sy, sx, sy], np.float32)
    x0 = np.clip(b[:, 0], np.float32(0.0), np.float32(img_w))
    y0 = np.clip(b[:, 1], np.float32(0.0), np.float32(img_h))
    x1 = np.clip(b[:, 2], np.float32(0.0), np.float32(img_w))
    y1 = np.clip(b[:, 3], np.float32(0.0), np.float32(img_h))
    return x0, y0, x1, y1


def _chunks(img_w):
    out = []
    c = 0
    while c < img_w:
        cw = min(512, img_w - c)
        out.append((c, cw))
        c += cw
    return out


def _build_win3(ni, img_h, img_w, bs):
    """bs: per-slot block counts (tuple of ni ints in [1, MAXB])."""
    import concourse.bass as bass
    import concourse.mybir as mybir
    from concourse import library_config
    from concourse.tile import TileContext

    f32 = mybir.dt.float32
    f32r = mybir.dt.float32r
    i32 = mybir.dt.int32
    B = sum(bs)
    nc = bass.Bass()
    w2t_d = nc.dram_tensor("w2t", [HM, RB * B], f32r, kind="ExternalInput")
    xw_d = nc.dram_tensor("xw", [HM, ni * WX], f32r, kind="ExternalInput")
    ctx_d = nc.dram_tensor("ctxidx", [128, ni], i32, kind="ExternalInput")
    out_d = nc.dram_tensor("out", [ni, img_h * img_w], f32, kind="ExternalOutput")

    with TileContext(nc) as tc:
        with (
            tc.tile_pool(name="w", bufs=1) as wp,
            tc.tile_pool(name="ix", bufs=1) as ixp,
            tc.tile_pool(name="ps", bufs=6, space="PSUM") as psp,
            tc.tile_pool(name="pay", bufs=4) as payp,
        ):
            nc.gpsimd.load_library(library_config.attn)
            idxs = ixp.tile([128, ni], i32, tag="idx")
            nc.sync.dma_start(out=idxs[:], in_=ctx_d[:])
            w2t = wp.tile([HM, RB * B], f32r, tag="w2t")
            xw = wp.tile([HM, ni * WX], f32r, tag="xw")
            nc.sync.dma_start(out=w2t[:], in_=w2t_d[:])
            nc.sync.dma_start(out=xw[:], in_=xw_d[:])
            off = 0
            for s in range(ni):
                b = bs[s]
                pay = payp.tile([128, b * WX], f32, tag=f"pay{b}")
                for k in range(b):
                    pb = psp.tile([128, WX], f32, tag="pb")
                    nc.tensor.matmul(
                        out=pb[:],
                        lhsT=w2t[:, (off + k) * RB : (off + k + 1) * RB],
                        rhs=xw[:, s * WX : (s + 1) * WX],
                        start=True,
                        stop=True,
                    )
                    eng = (
                        nc.vector.tensor_copy if (off + k) % 2 == 0 else nc.scalar.copy
                    )
                    eng(out=pay[:, k * WX : (k + 1) * WX], in_=pb[:])
                # out view [batch=1, dhi=128, dho=b, n_ctx]; row(p,k) = p*b+k,
                # byte addr = base + (p*b+k)*img_w*4 + ctx*4
                nctx = (img_h - RB * b + 1) * img_w
                base = out_d[s]
                out_ap = bass.AP(
                    base.tensor,
                    base.offset,
                    [[img_h * img_w, 1], [b * img_w, 128], [img_w, b], [1, nctx]],
                )
                in_ap = pay[:].rearrange("p (k w) -> p k w", k=b).unsqueeze(2)
                nc.gpsimd.kv_writeback(
                    out_ap=out_ap,
                    in_ap=in_ap,
                    ctx_idxs_ap=idxs[:, s : s + 1],
                    wraparound=False,
                )
                off += b
    from concourse.library_overlay import lower_extended_insts

    lower_extended_insts(nc)  # populate .instr for extended-ISA insts
    _split_multi_waits(nc)
    return nc


def _build_dense(ni, img_h, img_w):
    """Fallback: writes every output pixel (no window assumption)."""
    import concourse.bass as bass
    import concourse.mybir as mybir
    from concourse.tile import TileContext

    f32 = mybir.dt.float32
    f32r = mybir.dt.float32r
    nc = bass.Bass()
    maskT_d = nc.dram_tensor("maskT", [ni, WM, HM], f32r, kind="ExternalInput")
    x_d = nc.dram_tensor("xmat", [ni, WM, img_w], f32r, kind="ExternalInput")
    yt_d = nc.dram_tensor("ytmat", [ni, HM, img_h], f32r, kind="ExternalInput")
    out_d = nc.dram_tensor("out", [ni, img_h, img_w], f32, kind="ExternalOutput")
    chunks = _chunks(img_w)
    rtiles = []
    r = 0
    while r < img_h:
        rh = min(128, img_h - r)
        rtiles.append((r, rh))
        r += rh

    with TileContext(nc) as tc:
        with (
            tc.tile_pool(name="w", bufs=3) as wp,
            tc.tile_pool(name="mx", bufs=2) as mxp,
            tc.tile_pool(name="psA", bufs=2, space="PSUM") as psa,
            tc.tile_pool(name="psB", bufs=2, space="PSUM") as psb,
            tc.tile_pool(name="ob", bufs=4) as obp,
        ):
            for n in range(ni):
                mT = wp.tile([WM, HM], f32r, tag="mT")
                xt = wp.tile([WM, img_w], f32r, tag="xt")
                yt = wp.tile([HM, img_h], f32r, tag="yt")
                nc.sync.dma_start(out=mT[:], in_=maskT_d[n])
                nc.sync.dma_start(out=xt[:], in_=x_d[n])
                nc.sync.dma_start(out=yt[:], in_=yt_d[n])

                mx = mxp.tile([HM, img_w], f32r, tag="mx")
                for j, (c0, cw) in enumerate(chunks):
                    pa = psa.tile([HM, 512], f32, tag="pa")
                    nc.tensor.matmul(
                        out=pa[:, :cw], lhsT=mT[:], rhs=xt[:, c0 : c0 + cw],
                        start=True, stop=True,
                    )
                    if j % 2 == 0:
                        nc.vector.tensor_copy(out=mx[:, c0 : c0 + cw], in_=pa[:, :cw])
                    else:
                        nc.scalar.copy(out=mx[:, c0 : c0 + cw], in_=pa[:, :cw])

                for r0, rh in rtiles:
                    pb = psb.tile([128, 3 * 512], f32, tag="pb")
                    for k, (c0, cw) in enumerate(chunks):
                        nc.tensor.matmul(
                            out=pb[:rh, k * 512 : k * 512 + cw],
                            lhsT=yt[:, r0 : r0 + rh],
                            rhs=mx[:, c0 : c0 + cw],
                            start=True, stop=True,
                        )
                    ob = obp.tile([128, img_w], f32, tag="ob")
                    for k, (c0, cw) in enumerate(chunks):
                        eng = nc.vector.tensor_copy if k % 2 == 0 else nc.scalar.copy
                        eng(out=ob[:rh, c0 : c0 + cw], in_=pb[:rh, k * 512 : k * 512 + cw])
                    nc.sync.dma_start(out=out_d[n, r0 : r0 + rh, :], in_=ob[:rh, :])
    _split_multi_waits(nc)
    return nc


def _spans_starts(nzmask, size, budget):
    """Per-instance nonzero span and clamped window start for the given
    budget. Returns (first, span) arrays."""
    n = nzmask.shape[0]
    first = np.zeros(n, np.int64)
    span = np.zeros(n, np.int64)
    for i in range(n):
        nzr = np.flatnonzero(nzmask[i])
        if nzr.size:
            first[i] = int(nzr[0])
            span[i] = int(nzr[-1]) - int(nzr[0]) + 1
    return first, span


def _prep_win3(masks, xmat, ytmat, img_h, img_w, ni):
    n = masks.shape[0]
    rfirst, rspan = _spans_starts(ytmat.any(axis=1), img_h, RB * MAXB)
    cfirst, cspan = _spans_starts(xmat.any(axis=1), img_w, WX)
    if (
        rspan.max(initial=0) > RB * MAXB
        or cspan.max(initial=0) > WX
        or img_h < RB * MAXB
        or img_w < WX
    ):
        return None

    order = np.argsort(-rspan, kind="stable")  # rank r -> core r%8, slot r//8
    bs = []
    for s in range(ni):
        m = max(int(rspan[order[s * N_CORES]]), 1)
        bs.append(int(-(-m // RB)))
    bs = tuple(bs)

    # per-instance windows under its slot budget
    slot_of = np.empty(n, np.int64)
    core_of = np.empty(n, np.int64)
    for r, oid in enumerate(order):
        core_of[oid] = r % N_CORES
        slot_of[oid] = r // N_CORES
    budget = np.array([bs[slot_of[i]] * RB for i in range(n)], np.int64)
    r0 = np.minimum(rfirst, img_h - budget)
    c0 = np.minimum(cfirst, img_w - WX)

    # W2 = Y_w @ M over each instance's budgeted window, permuted so that
    # lhsT block k column p is window row p*b + k
    B = sum(bs)
    w2t_all = np.zeros((N_CORES, HM, RB * B), np.float32)
    xw_all = np.zeros((N_CORES, HM, ni * WX), np.float32)
    ctx_all = np.zeros((N_CORES, 128, ni), np.int32)
    offs = np.concatenate([[0], np.cumsum(bs)])
    for i in range(n):
        c, s = int(core_of[i]), int(slot_of[i])
        b = bs[s]
        rw = int(r0[i]) + np.arange(RB * b)
        ytw = ytmat[i][:, rw]                      # [28, 128b]
        w2 = masks[i, 0].T @ ytw                   # [28, 128b] (cols = window rows)
        perm = (np.arange(RB)[None, :] * b + np.arange(b)[:, None]).ravel()
        w2t_all[c, :, offs[s] * RB : offs[s + 1] * RB] = w2[:, perm]
        xw_all[c, :, s * WX : (s + 1) * WX] = xmat[i][
            :, int(c0[i]) : int(c0[i]) + WX
        ]
        ctx_all[c, :, s] = int(r0[i]) * img_w + int(c0[i])
    return bs, core_of, slot_of, w2t_all, xw_all, ctx_all


def _run(masks, boxes, img_h, img_w, in_h, in_w, trace=False):
    from concourse.bass_utils import run_bass_kernel_spmd

    n = masks.shape[0]
    assert n % N_CORES == 0
    ni = n // N_CORES
    x0, y0, x1, y1 = _scaled_boxes(boxes, img_h, img_w, in_h, in_w)
    xmat = _interp_mats(x0, x1, img_w, WM)   # [N, 28, img_w]
    ytmat = _interp_mats(y0, y1, img_h, HM)  # [N, 28, img_h]
    prep = _prep_win3(masks, xmat, ytmat, img_h, img_w, ni)

    if prep is not None:
        bs, core_of, slot_of, w2t_all, xw_all, ctx_all = prep
        key = ("win3", ni, img_h, img_w, bs)
        if key not in _BUILD_CACHE:
            _BUILD_CACHE[key] = _build_win3(ni, img_h, img_w, bs)
        nc = _BUILD_CACHE[key]
        in_maps = [
            {
                "w2t": np.ascontiguousarray(w2t_all[c]),
                "xw": np.ascontiguousarray(xw_all[c]),
                "ctxidx": np.ascontiguousarray(ctx_all[c]),
            }
            for c in range(N_CORES)
        ]
        res = run_bass_kernel_spmd(
            nc, in_maps, core_ids=list(range(N_CORES)), trace=trace
        )
        out = np.empty((n, img_h, img_w), np.float32)
        for i in range(n):
            out[i] = res.results[int(core_of[i])]["out"][int(slot_of[i])].reshape(
                img_h, img_w
            )
        return out, res

    key = ("dense", ni, img_h, img_w)
    if key not in _BUILD_CACHE:
        _BUILD_CACHE[key] = _build_dense(ni, img_h, img_w)
    nc = _BUILD_CACHE[key]
    maskt = np.ascontiguousarray(
        np.transpose(masks[:, 0].astype(np.float32), (0, 2, 1))
    )
    in_maps = []
    for c in range(N_CORES):
        s = slice(c * ni, (c + 1) * ni)
        in_maps.append({"maskT": maskt[s], "xmat": xmat[s], "ytmat": ytmat[s]})
    res = run_bass_kernel_spmd(nc, in_maps, core_ids=list(range(N_CORES)), trace=trace)
    out = np.concatenate([res.results[c]["out"] for c in range(N_CORES)], axis=0)
    return out, res


def kernel(masks, boxes, img_h, img_w, in_h, in_w):
    img_h, img_w, in_h, in_w = int(img_h), int(img_w), int(in_h), int(in_w)
    masks = np.asarray(masks, dtype=np.float32)
    boxes = np.asarray(boxes, dtype=np.float32)
    out, _ = _run(masks, boxes, img_h, img_w, in_h, in_w, trace=False)
    return out


# revision 14
# speedup vs baseline: 3.2629x; 1.1141x over previous
"""Mask R-CNN paste_masks_in_image kernel for Trainium2 (8 NeuronCores).

out[n] = Y_n @ mask_n @ X_n  (separable bilinear paste, f32)

Fast path (windowed, variable budgets): host folds W2_n = (Y_n @ M_n)
over the instance's row window and slices X_n to a per-slot column
window. Instances are sorted by (row-blocks desc, col-span desc) and
dealt round-robin so all 8 cores share one slot->budget pattern
(b blocks of 128 rows; ncn cols, pow2 or <256). Consecutive same-b
slot pairs share one batched kv_writeback whose int32 ctx indices
carry the dynamic flat offsets r0*img_w + c0. Inputs are bf16 (PSUM
accumulates f32; tol is 2e-2). Rows/cols outside windows are never
written: the runner pre-zeros/donates output buffers.

Falls back to a dense full-image writer if any window exceeds the
static budgets (cannot happen for in-distribution inputs).
"""
import sys

if "/opt/trn_rl_repo" not in sys.path:
    sys.path.insert(0, "/opt/trn_rl_repo")

import numpy as np

N_CORES = 8
HM = WM = 28
RB = 128          # rows per block (= partitions per matmul)
MAXB = 3          # max blocks per slot -> max row span 384
MAXW = 512        # max column window

_BUILD_CACHE = {}
_ws_ctr = [0]


def _split_multi_waits(nc):
    """This image's walrus allows only ONE sync-wait per instruction; hoist
    extra waits onto preceding NoOps on the same engine."""
    import concourse.mybir as mybir

    for fn in nc.m.functions:
        for blk in fn.blocks:
            insts = list(blk.instructions)
            out = []
            changed = False
            for inst in insts:
                si = getattr(inst, "sync_info", None)
                waits = list(si.on_wait) if (si is not None and si.on_wait) else []
                if len(waits) > 1:
                    changed = True
                    for w in waits[:-1]:
                        _ws_ctr[0] += 1
                        out.append(
                            mybir.InstNoOp(
                                name=f"waitsplit-{_ws_ctr[0]}",
                                engine=inst.engine,
                                sync_info=mybir.SyncInfo(on_wait=[w], on_update=[]),
                            )
                        )
                    si.on_wait = [waits[-1]]
                out.append(inst)
            if changed:
                try:
                    blk.instructions = out
                except Exception:
                    del blk.instructions[:]
                    blk.instructions.extend(out)


def _interp_mats(p0, p1, out_size, mask_size):
    """W[n, k, j] = w0*(i0==k) + w1*(i0+1==k); exact f32 replication of the
    reference's align_corners=False bilinear weights with zero padding."""
    xs = (np.arange(out_size, dtype=np.float32) + np.float32(0.5))[None, :]
    g = (xs - p0[:, None]) / (p1 - p0)[:, None] * np.float32(2) - np.float32(1)
    p = (g + np.float32(1)) * np.float32(mask_size * 0.5) - np.float32(0.5)
    f = np.floor(p)
    i0 = f.astype(np.int64)
    w1 = (p - f).astype(np.float32)
    w0 = np.float32(1.0) - w1
    ks = np.arange(mask_size, dtype=np.int64)[None, :, None]
    W = (i0[:, None, :] == ks) * w0[:, None, :] + ((i0 + 1)[:, None, :] == ks) * w1[
        :, None, :
    ]
    return np.ascontiguousarray(W.astype(np.float32))


def _scaled_boxes(boxes, img_h, img_w, in_h, in_w):
    sx = np.float32(img_w / in_w)
    sy = np.float32(img_h / in_h)
    b = boxes.astype(np.float32) * np.array([sx, sy, sx, sy], np.float32)
    x0 = np.clip(b[:, 0], np.float32(0.0), np.float32(img_w))
    y0 = np.clip(b[:, 1], np.float32(0.0), np.float32(img_h))
    x1 = np.clip(b[:, 2], np.float32(0.0), np.float32(img_w))
    y1 = np.clip(b[:, 3], np.float32(0.0), np.float32(img_h))
    return x0, y0, x1, y1


def _chunks(img_w):
    out = []
    c = 0
    while c < img_w:
        cw = min(512, img_w - c)
        out.append((c, cw))
        c += cw
    return out


def _build_win4(ni, img_h, img_w, groups):
    """groups: tuple of (b, nb, ncn) covering slots in order; one batched
    kv_writeback per group."""
    import concourse.bass as bass
    import concourse.mybir as mybir
    from concourse import library_config
    from concourse.tile import TileContext

    f32 = mybir.dt.float32
    bf16 = mybir.dt.bfloat16
    i32 = mybir.dt.int32
    Btot = sum(g[0] * g[1] for g in groups)       # total 128-row blocks
    Xtot = sum(g[1] * g[2] for g in groups)       # total xw columns
    # split load halves at a group boundary near half the blocks
    acc = 0
    g_half = len(groups)
    for gi, g in enumerate(groups):
        acc += g[0] * g[1]
        if acc >= Btot // 2:
            g_half = gi + 1
            break
    wsplit = sum(g[0] * g[1] for g in groups[:g_half])
    xsplit = sum(g[1] * g[2] for g in groups[:g_half])

    nc = bass.Bass()
    w2t_d = nc.dram_tensor("w2t", [HM, RB * Btot], bf16, kind="ExternalInput")
    xw_d = nc.dram_tensor("xw", [HM, Xtot], bf16, kind="ExternalInput")
    ctx_d = nc.dram_tensor("ctxidx", [128, ni], i32, kind="ExternalInput")
    out_d = nc.dram_tensor("out", [ni, img_h * img_w], f32, kind="ExternalOutput")

    with TileContext(nc) as tc:
        with (
            tc.tile_pool(name="w", bufs=1) as wp,
            tc.tile_pool(name="ix", bufs=1) as ixp,
            tc.tile_pool(name="ps", bufs=6, space="PSUM") as psp,
            tc.tile_pool(name="pay", bufs=3) as payp,
        ):
            nc.gpsimd.load_library(library_config.attn)
            idxs = ixp.tile([128, ni], i32, tag="idx")
            nc.sync.dma_start(out=idxs[:], in_=ctx_d[:])
            wh = [
                wp.tile([HM, RB * wsplit], bf16, tag="w2tA", name="w2tA"),
                wp.tile([HM, RB * (Btot - wsplit)], bf16, tag="w2tB", name="w2tB"),
            ]
            xh = [
                wp.tile([HM, xsplit], bf16, tag="xwA", name="xwA"),
                wp.tile([HM, Xtot - xsplit], bf16, tag="xwB", name="xwB"),
            ]
            nc.sync.dma_start(out=wh[0][:], in_=w2t_d[:, : RB * wsplit])
            nc.sync.dma_start(out=xh[0][:], in_=xw_d[:, :xsplit])
            nc.sync.dma_start(out=wh[1][:], in_=w2t_d[:, RB * wsplit :])
            nc.sync.dma_start(out=xh[1][:], in_=xw_d[:, xsplit:])

            s = 0
            off_w = 0   # block offset into w2t
            off_x = 0   # col offset into xw
            paymax = max(g[0] * g[1] * g[2] for g in groups)
            for gi, (b, nb, ncn) in enumerate(groups):
                h = 0 if gi < g_half else 1
                ow = off_w - (0 if h == 0 else wsplit)
                ox = off_x - (0 if h == 0 else xsplit)
                payb = payp.tile([128, paymax], f32, tag="pay", name="payt")
                pay = payb[:, : b * nb * ncn]
                for j in range(nb):
                    eng = (
                        nc.vector.tensor_copy
                        if (s + j) % 2 == 0
                        else nc.scalar.copy
                    )
                    for k in range(b):
                        pb = psp.tile([128, 512], f32, tag="pb", name="pbt")
                        nc.tensor.matmul(
                            out=pb[:, :ncn],
                            lhsT=wh[h][:, (ow + j * b + k) * RB : (ow + j * b + k + 1) * RB],
                            rhs=xh[h][:, ox + j * ncn : ox + (j + 1) * ncn],
                            start=True,
                            stop=True,
                        )
                        eng(
                            out=pay[:, (k * nb + j) * ncn : (k * nb + j + 1) * ncn],
                            in_=pb[:, :ncn],
                        )
                nctx = (img_h - RB * b + 1) * img_w
                base = out_d[s]
                out_ap = bass.AP(
                    base.tensor,
                    base.offset,
                    [[img_h * img_w, nb], [b * img_w, 128], [img_w, b], [1, nctx]],
                )
                in_ap = pay[:].rearrange("p (k j w) -> p k j w", k=b, j=nb)
                nc.gpsimd.kv_writeback(
                    out_ap=out_ap,
                    in_ap=in_ap,
                    ctx_idxs_ap=idxs[:, s : s + nb],
                    wraparound=False,
                )
                s += nb
                off_w += nb * b
                off_x += nb * ncn
    from concourse.library_overlay import lower_extended_insts

    lower_extended_insts(nc)  # populate .instr for extended-ISA insts
    _split_multi_waits(nc)
    return nc


def _build_win5(ni, img_h, img_w, groups):
    """Like _build_win4 but with prepare_only kv_writebacks emitted up
    front (descriptor gen off the critical path; needs only the ctx-index
    DMA) and a cheap per-group trigger_dma after the PSUM->SBUF copies. A
    1-row gpsimd dummy read of each pay tile carries the copies->trigger
    dependency that Tile does not thread through bare triggers."""
    import concourse.bass as bass
    import concourse.mybir as mybir
    from concourse import library_config
    from concourse.tile import TileContext

    f32 = mybir.dt.float32
    bf16 = mybir.dt.bfloat16
    i32 = mybir.dt.int32
    Btot = sum(g[0] * g[1] for g in groups)
    Xtot = sum(g[1] * g[2] for g in groups)
    acc = 0
    g_half = len(groups)
    for gi, g in enumerate(groups):
        acc += g[0] * g[1]
        if acc >= Btot // 2:
            g_half = gi + 1
            break
    wsplit = sum(g[0] * g[1] for g in groups[:g_half])
    xsplit = sum(g[1] * g[2] for g in groups[:g_half])

    nc = bass.Bass()
    w2t_d = nc.dram_tensor("w2t", [HM, RB * Btot], bf16, kind="ExternalInput")
    xw_d = nc.dram_tensor("xw", [HM, Xtot], bf16, kind="ExternalInput")
    ctx_d = nc.dram_tensor("ctxidx", [128, ni], i32, kind="ExternalInput")
    out_d = nc.dram_tensor("out", [ni, img_h * img_w], f32, kind="ExternalOutput")
    dma_sem = nc.alloc_semaphore("kvdma")

    with TileContext(nc) as tc:
        with (
            tc.tile_pool(name="w", bufs=1) as wp,
            tc.tile_pool(name="ix", bufs=1) as ixp,
            tc.tile_pool(name="ps", bufs=6, space="PSUM") as psp,
            tc.tile_pool(name="pay", bufs=3) as payp,
            tc.tile_pool(name="dr", bufs=1) as drp,
        ):
            nc.gpsimd.load_library(library_config.attn)
            idxs = ixp.tile([128, ni], i32, tag="idx")
            nc.sync.dma_start(out=idxs[:], in_=ctx_d[:])
            wh = [
                wp.tile([HM, RB * wsplit], bf16, tag="w2tA", name="w2tA"),
                wp.tile([HM, RB * (Btot - wsplit)], bf16, tag="w2tB", name="w2tB"),
            ]
            xh = [
                wp.tile([HM, xsplit], bf16, tag="xwA", name="xwA"),
                wp.tile([HM, Xtot - xsplit], bf16, tag="xwB", name="xwB"),
            ]
            nc.sync.dma_start(out=wh[0][:], in_=w2t_d[:, : RB * wsplit])
            nc.sync.dma_start(out=xh[0][:], in_=xw_d[:, :xsplit])
            nc.sync.dma_start(out=wh[1][:], in_=w2t_d[:, RB * wsplit :])
            nc.sync.dma_start(out=xh[1][:], in_=xw_d[:, xsplit:])

            paymax = max(g[0] * g[1] * g[2] for g in groups)
            scr = drp.tile([1, paymax], f32, tag="scr", name="scr")
            # allocate all pay tiles and emit all preps first (ring order)
            pays = []
            preps = []
            s = 0
            for gi, (b, nb, ncn) in enumerate(groups):
                payb = payp.tile([128, paymax], f32, tag="pay", name="payt")
                pay = payb[:, : b * nb * ncn]
                pays.append(pay)
                nctx = (img_h - RB * b + 1) * img_w
                base = out_d[s]
                out_ap = bass.AP(
                    base.tensor,
                    base.offset,
                    [[img_h * img_w, nb], [b * img_w, 128], [img_w, b], [1, nctx]],
                )
                in_ap = pay.rearrange("p (k j w) -> p k j w", k=b, j=nb)
                preps.append(
                    nc.gpsimd.kv_writeback(
                        out_ap=out_ap,
                        in_ap=in_ap,
                        ctx_idxs_ap=idxs[:, s : s + nb],
                        wraparound=False,
                        prepare_only=True,
                        sem=dma_sem,
                    )
                )
                s += nb

            s = 0
            off_w = 0
            off_x = 0
            prev_trig = None
            for gi, (b, nb, ncn) in enumerate(groups):
                h = 0 if gi < g_half else 1
                ow = off_w - (0 if h == 0 else wsplit)
                ox = off_x - (0 if h == 0 else xsplit)
                pay = pays[gi]
                for j in range(nb):
                    eng = (
                        nc.vector.tensor_copy
                        if (s + j) % 2 == 0
                        else nc.scalar.copy
                    )
                    for k in range(b):
                        pb = psp.tile([128, 512], f32, tag="pb", name="pbt")
                        nc.tensor.matmul(
                            out=pb[:, :ncn],
                            lhsT=wh[h][:, (ow + j * b + k) * RB : (ow + j * b + k + 1) * RB],
                            rhs=xh[h][:, ox + j * ncn : ox + (j + 1) * ncn],
                            start=True,
                            stop=True,
                        )
                        eng(
                            out=pay[:, (k * nb + j) * ncn : (k * nb + j + 1) * ncn],
                            in_=pb[:, :ncn],
                        )
                # dummy gpsimd read of pay row 0 -> Tile makes the trigger
                # (next Pool inst, in order) safe w.r.t. the copies
                dum = nc.gpsimd.tensor_copy(
                    out=scr[:, : b * nb * ncn], in_=pay[0:1, :]
                )
                trig = nc.gpsimd.trigger_dma(count=1)
                deps = {dum.ins.name, preps[gi].ins.name}
                if prev_trig is not None:
                    deps.add(prev_trig.ins.name)
                trig.ins.add_nosync_dependencies_from(deps)
                prev_trig = trig
                s += nb
                off_w += nb * b
                off_x += nb * ncn
    from concourse.library_overlay import lower_extended_insts

    lower_extended_insts(nc)
    _split_multi_waits(nc)
    return nc


def _build_dense(ni, img_h, img_w):
    """Fallback: writes every output pixel (no window assumption)."""
    import concourse.bass as bass
    import concourse.mybir as mybir
    from concourse.tile import TileContext

    f32 = mybir.dt.float32
    f32r = mybir.dt.float32r
    nc = bass.Bass()
    maskT_d = nc.dram_tensor("maskT", [ni, WM, HM], f32r, kind="ExternalInput")
    x_d = nc.dram_tensor("xmat", [ni, WM, img_w], f32r, kind="ExternalInput")
    yt_d = nc.dram_tensor("ytmat", [ni, HM, img_h], f32r, kind="ExternalInput")
    out_d = nc.dram_tensor("out", [ni, img_h, img_w], f32, kind="ExternalOutput")
    chunks = _chunks(img_w)
    rtiles = []
    r = 0
    while r < img_h:
        rh = min(128, img_h - r)
        rtiles.append((r, rh))
        r += rh

    with TileContext(nc) as tc:
        with (
            tc.tile_pool(name="w", bufs=3) as wp,
            tc.tile_pool(name="mx", bufs=2) as mxp,
            tc.tile_pool(name="psA", bufs=2, space="PSUM") as psa,
            tc.tile_pool(name="psB", bufs=2, space="PSUM") as psb,
            tc.tile_pool(name="ob", bufs=4) as obp,
        ):
            for n in range(ni):
                mT = wp.tile([WM, HM], f32r, tag="mT")
                xt = wp.tile([WM, img_w], f32r, tag="xt")
                yt = wp.tile([HM, img_h], f32r, tag="yt")
                nc.sync.dma_start(out=mT[:], in_=maskT_d[n])
                nc.sync.dma_start(out=xt[:], in_=x_d[n])
                nc.sync.dma_start(out=yt[:], in_=yt_d[n])

                mx = mxp.tile([HM, img_w], f32r, tag="mx")
                for j, (c0, cw) in enumerate(chunks):
                    pa = psa.tile([HM, 512], f32, tag="pa")
                    nc.tensor.matmul(
                        out=pa[:, :cw], lhsT=mT[:], rhs=xt[:, c0 : c0 + cw],
                        start=True, stop=True,
                    )
                    if j % 2 == 0:
                        nc.vector.tensor_copy(out=mx[:, c0 : c0 + cw], in_=pa[:, :cw])
                    else:
                        nc.scalar.copy(out=mx[:, c0 : c0 + cw], in_=pa[:, :cw])

                for r0, rh in rtiles:
                    pb = psb.tile([128, 3 * 512], f32, tag="pb")
                    for k, (c0, cw) in enumerate(chunks):
                        nc.tensor.matmul(
                            out=pb[:rh, k * 512 : k * 512 + cw],
                            lhsT=yt[:, r0 : r0 + rh],
                            rhs=mx[:, c0 : c0 + cw],
                            start=True, stop=True,
                        )
                    ob = obp.tile([128, img_w], f32, tag="ob")
                    for k, (c0, cw) in enumerate(chunks):
                        eng = nc.vector.tensor_copy if k % 2 == 0 else nc.scalar.copy
                        eng(out=ob[:rh, c0 : c0 + cw], in_=pb[:rh, k * 512 : k * 512 + cw])
                    nc.sync.dma_start(out=out_d[n, r0 : r0 + rh, :], in_=ob[:rh, :])
    _split_multi_waits(nc)
    return nc


def _spans(nzmask):
    n = nzmask.shape[0]
    first = np.zeros(n, np.int64)
    span = np.zeros(n, np.int64)
    for i in range(n):
        nzr = np.flatnonzero(nzmask[i])
        if nzr.size:
            first[i] = int(nzr[0])
            span[i] = int(nzr[-1]) - int(nzr[0]) + 1
    return first, span


def _prep_win4(masks, xmat, ytmat, img_h, img_w, ni):
    import ml_dtypes

    n = masks.shape[0]
    rfirst, rspan = _spans(ytmat.any(axis=1))
    cfirst, cspan = _spans(xmat.any(axis=1))
    if (
        rspan.max(initial=0) > RB * MAXB
        or cspan.max(initial=0) > MAXW
        or img_h < RB * MAXB
        or img_w < MAXW
    ):
        return None

    b_inst = np.maximum(-(-rspan // RB), 1)
    order = np.lexsort((-cspan, -b_inst))   # rank r -> core r%8, slot r//8
    core_of = np.empty(n, np.int64)
    slot_of = np.empty(n, np.int64)
    for r, oid in enumerate(order):
        core_of[oid] = r % N_CORES
        slot_of[oid] = r // N_CORES
    bs, cmax = [], []
    for s in range(ni):
        grp = order[s * N_CORES : (s + 1) * N_CORES]
        bs.append(int(b_inst[grp].max()))
        cmax.append(int(cspan[grp].max()))

    # pair consecutive same-b slots into batched writebacks
    groups = []          # (b, nb, ncn)
    slot_ncn = [0] * ni
    s = 0
    while s < ni:
        nb = 2 if (s + 1 < ni and bs[s + 1] == bs[s]) else 1
        c = max(cmax[s : s + nb])
        ncn = min(-(-c // 32) * 32, MAXW)
        if ncn > 256:
            ncn = MAXW
        ncn = max(ncn, 32)
        for j in range(nb):
            slot_ncn[s + j] = ncn
        groups.append((bs[s], nb, ncn))
        s += nb
    groups = tuple(groups)

    budget = np.array([bs[slot_of[i]] * RB for i in range(n)], np.int64)
    width = np.array([slot_ncn[slot_of[i]] for i in range(n)], np.int64)
    r0 = np.minimum(rfirst, img_h - budget)
    c0 = np.minimum(cfirst, img_w - width)

    Btot = sum(g[0] * g[1] for g in groups)
    Xtot = sum(g[1] * g[2] for g in groups)
    woff = [0] * ni   # per-slot block offset into w2t
    xoff = [0] * ni   # per-slot col offset into xw
    s = 0
    ow = ox = 0
    for b, nb, ncn in groups:
        for j in range(nb):
            woff[s + j] = ow + j * b
            xoff[s + j] = ox + j * ncn
        s += nb
        ow += nb * b
        ox += nb * ncn

    bf = ml_dtypes.bfloat16
    w2t_all = np.zeros((N_CORES, HM, RB * Btot), bf)
    xw_all = np.zeros((N_CORES, HM, Xtot), bf)
    ctx_all = np.zeros((N_CORES, 128, ni), np.int32)
    for i in range(n):
        c, s = int(core_of[i]), int(slot_of[i])
        b = bs[s]
        ncn = slot_ncn[s]
        rw = int(r0[i]) + np.arange(RB * b)
        ytw = ytmat[i][:, rw]                      # [28, 128b]
        w2 = masks[i, 0].T @ ytw                   # [28, 128b] (cols = window rows)
        perm = (np.arange(RB)[None, :] * b + np.arange(b)[:, None]).ravel()
        w2t_all[c, :, woff[s] * RB : (woff[s] + b) * RB] = w2[:, perm].astype(bf)
        xw_all[c, :, xoff[s] : xoff[s] + ncn] = xmat[i][
            :, int(c0[i]) : int(c0[i]) + ncn
        ].astype(bf)
        ctx_all[c, :, s] = int(r0[i]) * img_w + int(c0[i])
    return groups, core_of, slot_of, w2t_all, xw_all, ctx_all


def _run(masks, boxes, img_h, img_w, in_h, in_w, trace=False):
    from concourse.bass_utils import run_bass_kernel_spmd

    n = masks.shape[0]
    assert n % N_CORES == 0
    ni = n // N_CORES
    x0, y0, x1, y1 = _scaled_boxes(boxes, img_h, img_w, in_h, in_w)
    xmat = _interp_mats(x0, x1, img_w, WM)   # [N, 28, img_w]
    ytmat = _interp_mats(y0, y1, img_h, HM)  # [N, 28, img_h]
    prep = _prep_win4(masks, xmat, ytmat, img_h, img_w, ni)

    if prep is not None:
        groups, core_of, slot_of, w2t_all, xw_all, ctx_all = prep
        key = ("win4", ni, img_h, img_w, groups)
        if key not in _BUILD_CACHE:
            _BUILD_CACHE[key] = _build_win4(ni, img_h, img_w, groups)
        nc = _BUILD_CACHE[key]
        in_maps = [
            {
                "w2t": np.ascontiguousarray(w2t_all[c]),
                "xw": np.ascontiguousarray(xw_all[c]),
                "ctxidx": np.ascontiguousarray(ctx_all[c]),
            }
            for c in range(N_CORES)
        ]
        res = run_bass_kernel_spmd(
            nc, in_maps, core_ids=list(range(N_CORES)), trace=trace
        )
        out = np.empty((n, img_h, img_w), np.float32)
        for i in range(n):
            out[i] = res.results[int(core_of[i])]["out"][int(slot_of[i])].reshape(
                img_h, img_w
            )
        return out, res

    key = ("dense", ni, img_h, img_w)
    if key not in _BUILD_CACHE:
        _BUILD_CACHE[key] = _build_dense(ni, img_h, img_w)
    nc = _BUILD_CACHE[key]
    maskt = np.ascontiguousarray(
        np.transpose(masks[:, 0].astype(np.float32), (0, 2, 1))
    )
    in_maps = []
    for c in range(N_CORES):
        s = slice(c * ni, (c + 1) * ni)
        in_maps.append({"maskT": maskt[s], "xmat": xmat[s], "ytmat": ytmat[s]})
    res = run_bass_kernel_spmd(nc, in_maps, core_ids=list(range(N_CORES)), trace=trace)
    out = np.concatenate([res.results[c]["out"] for c in range(N_CORES)], axis=0)
    return out, res


def kernel(masks, boxes, img_h, img_w, in_h, in_w):
    img_h, img_w, in_h, in_w = int(img_h), int(img_w), int(in_h), int(in_w)
    masks = np.asarray(masks, dtype=np.float32)
    boxes = np.asarray(boxes, dtype=np.float32)
    out, _ = _run(masks, boxes, img_h, img_w, in_h, in_w, trace=False)
    return out
